# revision 12
# baseline (speedup 1.0000x reference)
"""Trainium2 Bass kernel for the AttnRNN cell.

Data-parallel over batch across 8 NeuronCores (512 rows each).  All 15
[512,1024]x[1024,1024] GEMMs run in bf16 with fp32 PSUM accumulation;
elementwise state math stays fp32.  Activations are kept in transposed
[feature, batch] layout (TensorE contracts over the partition dim), with
host-side pre-transposition of x/hiddens so no on-chip input transposes
are needed.
"""

import sys

for _p in ("/opt/trn_rl_repo",):
    if _p not in sys.path:
        sys.path.append(_p)

import numpy as np
import ml_dtypes

import concourse.bass as bass
import concourse.mybir as mybir
import concourse.tile as tile
from concourse import bacc
from concourse.bass_utils import run_bass_kernel_spmd
from concourse.masks import make_identity

BF16 = mybir.dt.bfloat16
F32 = mybir.dt.float32
AF = mybir.ActivationFunctionType
ALU = mybir.AluOpType

B, D, H, K, A = 4096, 1024, 1024, 8, 8
NCORES = 8
BS = B // NCORES          # 512 batch rows per core
P = 128                   # partitions
NT = BS // P              # 4 batch tiles per core
JT = D // P               # 8 contraction tiles
bf16 = ml_dtypes.bfloat16

_CACHE = {}


def _build():
    nc = bacc.Bacc("TRN2", target_bir_lowering=False, debug=False,
                   num_devices=NCORES)

    dram = {}

    def din(name, shape, dt):
        dram[name] = nc.dram_tensor(name, list(shape), dt, kind="ExternalInput")
        return dram[name]

    din("xT", (D, BS), BF16)                    # x shard, transposed
    din("hT", (K, H, BS), BF16)                 # hiddens shard, transposed
    din("cl", (BS, H), F32)                     # cells[-1] shard, natural
    for w in ("Wfx", "Wox", "Wix", "Wux", "Wfh", "Woh", "Wih"):
        din(w, (D, H), BF16)
    din("Wk", (K, H, H), BF16)
    din("attnW", (H, A), BF16)
    din("attnWu", (A, 1), BF16)
    din("attnb", (A, 1), F32)
    din("bI", (P, JT), F32)                     # bix+bih, [128, h_tile]
    din("bF", (P, JT), F32)
    din("bO", (P, JT), F32)
    din("bU", (P, JT), F32)
    din("bkr", (P, K, JT), F32)                 # bk, [128, k, o_tile]
    din("ones", (1, P), BF16)

    hid_o = nc.dram_tensor("hidden", [BS, H], F32, kind="ExternalOutput")
    cel_o = nc.dram_tensor("cell", [BS, H], F32, kind="ExternalOutput")

    with tile.TileContext(nc) as tc:
        _body(nc, tc, dram, hid_o, cel_o)
    nc.compile()
    return nc


def _body(nc, tc, dram, hid_o, cel_o):
    from contextlib import ExitStack
    ctx = ExitStack()
    with ctx:
        cpool = ctx.enter_context(tc.tile_pool(name="consts", bufs=1))
        wpool = ctx.enter_context(tc.tile_pool(name="w", bufs=3))
        hpool = ctx.enter_context(tc.tile_pool(name="ht", bufs=2))
        gpool = ctx.enter_context(tc.tile_pool(name="g", bufs=3))
        big_p = ctx.enter_context(tc.tile_pool(name="big", bufs=1))
        ua_p = ctx.enter_context(tc.tile_pool(name="uatt", bufs=2))
        sm_p = ctx.enter_context(tc.tile_pool(name="smallf", bufs=2))
        pr_p = ctx.enter_context(tc.tile_pool(name="prod", bufs=1))
        nf_p = ctx.enter_context(tc.tile_pool(name="natf", bufs=2))
        cl_p = ctx.enter_context(tc.tile_pool(name="clp", bufs=2))
        out_p = ctx.enter_context(tc.tile_pool(name="outp", bufs=3))
        tmp_p = ctx.enter_context(tc.tile_pool(name="tmpp", bufs=2))
        ps = ctx.enter_context(tc.tile_pool(name="ps", bufs=8, space="PSUM"))

        # ---- constants / resident inputs ----
        xT_sb = cpool.tile([P, JT, BS], BF16)
        nc.sync.dma_start(xT_sb[:], dram["xT"].ap().rearrange("(j p) b -> p j b", p=P))
        h7_sb = cpool.tile([P, JT, BS], BF16)
        nc.sync.dma_start(h7_sb[:], dram["hT"].ap()[K - 1].rearrange("(j p) b -> p j b", p=P))
        attnW_sb = cpool.tile([P, JT, A], BF16)
        nc.sync.dma_start(attnW_sb[:], dram["attnW"].ap().rearrange("(j p) a -> p j a", p=P))
        attnWu_sb = cpool.tile([A, 1], BF16)
        nc.sync.dma_start(attnWu_sb[:], dram["attnWu"].ap()[:])
        attnb_sb = cpool.tile([A, 1], F32)
        nc.sync.dma_start(attnb_sb[:], dram["attnb"].ap()[:])
        ones_sb = cpool.tile([1, P], BF16)
        nc.sync.dma_start(ones_sb[:], dram["ones"].ap()[:])
        bias_sb = {}
        for nm in ("bI", "bF", "bO", "bU"):
            bias_sb[nm] = cpool.tile([P, JT], F32, name=nm, tag=nm)
            nc.sync.dma_start(bias_sb[nm][:], dram[nm].ap()[:])
        bkr_sb = cpool.tile([P, K, JT], F32)
        nc.sync.dma_start(bkr_sb[:], dram["bkr"].ap()[:])
        id_bf = cpool.tile([P, P], BF16)
        make_identity(nc, id_bf[:])

        # persistent tensors (bufs=1 pool); i_gt's slot is reused by abc
        # (i_gt is dead before abc is written)
        i_gt = big_p.tile([P, JT, BS], BF16, tag="sh8")
        hs = big_p.tile([P, JT, BS, K], BF16, tag="hs")   # [p, o_tile, b, k]
        ex_f = big_p.tile([1, BS, K], BF16, tag="exf")    # exp(scores), one row
        al_f = big_p.tile([1, BS, K], BF16, tag="alf")    # alphas, single row
        fT = big_p.tile([P, JT, BS], BF16, tag="fT")
        oT = big_p.tile([P, JT, BS], BF16, tag="oT")
        utT = big_p.tile([P, JT, BS], BF16, tag="utT")

        def gate_gemm(wx_name, wh_name):
            """psums[i] = x@Wx[:,i] + h7@Wh[:,i] for each h-tile i (T-land)."""
            psl = [ps.tile([P, BS], F32, name=f"psg{i}", tag="ps")
                   for i in range(JT)]
            for j in range(JT):
                wt = wpool.tile([P, H], BF16, tag="w")
                nc.sync.dma_start(wt[:], dram[wx_name].ap()[j * P:(j + 1) * P, :])
                for i in range(JT):
                    nc.tensor.matmul(psl[i][:], wt[:, i * P:(i + 1) * P],
                                     xT_sb[:, j, :], start=(j == 0), stop=False)
            for j in range(JT):
                wt = wpool.tile([P, H], BF16, tag="w")
                nc.sync.dma_start(wt[:], dram[wh_name].ap()[j * P:(j + 1) * P, :])
                for i in range(JT):
                    nc.tensor.matmul(psl[i][:], wt[:, i * P:(i + 1) * P],
                                     h7_sb[:, j, :], start=False, stop=(j == JT - 1))
            return psl

        # ---- I gate (first: i_gt feeds everything) ----
        psl = gate_gemm("Wix", "Wih")
        for i in range(JT):
            nc.scalar.activation(i_gt[:, i, :], psl[i][:], AF.Sigmoid,
                                 bias=bias_sb["bI"][:, i:i + 1])

        # ---- per-step gated projections hs[k] + attention scores ----
        for k in range(K):
            psl = [ps.tile([P, BS], F32, name=f"psk{i}", tag="ps")
                   for i in range(JT)]
            for j in range(JT):
                ht = hpool.tile([P, BS], BF16, tag="ht")
                nc.sync.dma_start(ht[:], dram["hT"].ap()[k, j * P:(j + 1) * P, :])
                g = gpool.tile([P, BS], BF16, tag="g")
                nc.vector.tensor_tensor(g[:], ht[:], i_gt[:, j, :], ALU.mult)
                wt = wpool.tile([P, H], BF16, tag="w")
                nc.sync.dma_start(wt[:], dram["Wk"].ap()[k, j * P:(j + 1) * P, :])
                for i in range(JT):
                    nc.tensor.matmul(psl[i][:], wt[:, i * P:(i + 1) * P],
                                     g[:], start=(j == 0), stop=(j == JT - 1))
            for i in range(JT):
                nc.scalar.activation(hs[:, i, :, k], psl[i][:], AF.Identity,
                                     bias=bkr_sb[:, k, i:i + 1])
            # u_att[k] = tanh(hs[k] @ attnW + attnb)  -> [A, BS]
            ps_ua = ps.tile([A, BS], F32, tag="ps")
            for j in range(JT):
                nc.tensor.matmul(ps_ua[:], attnW_sb[:, j, :], hs[:, j, :, k],
                                 start=(j == 0), stop=(j == JT - 1))
            ua = ua_p.tile([A, BS], BF16, tag="ua")
            nc.scalar.activation(ua[:], ps_ua[:], AF.Tanh, bias=attnb_sb[:])
            # uv[k, :] = attnWu . u_att[k]
            ps_uv = ps.tile([1, BS], F32, tag="ps")
            nc.tensor.matmul(ps_uv[:], attnWu_sb[:], ua[:], start=True, stop=True)
            nc.scalar.activation(ex_f[:, :, k], ps_uv[:], AF.Exp)

        # ---- softmax over k (single-partition row, k innermost) ----
        sume = sm_p.tile([1, BS], F32, tag="sume", bufs=1)
        nc.vector.tensor_reduce(sume[:], ex_f[:], mybir.AxisListType.X, ALU.add)
        rec = sm_p.tile([1, BS], F32, tag="rec", bufs=1)
        nc.vector.reciprocal(rec[:], sume[:])
        nc.vector.tensor_tensor(al_f[:], ex_f[:],
                                rec[:, :, None].to_broadcast((1, BS, K)),
                                ALU.mult)

        # ---- broadcast alphas over partitions: abc[p, b, k] = alpha[k, b] ----
        abc = big_p.tile([P, BS, K], BF16, tag="sh8")
        CH = 512
        nch = BS * K // CH
        al_v = al_f[:].rearrange("o (c x) k -> o c (x k)", x=CH // K)
        abc_v = abc[:].rearrange("p (c x) k -> p c (x k)", x=CH // K)
        for c in range(nch):
            ps_b = ps.tile([P, CH], F32, tag="ps")
            nc.tensor.matmul(ps_b[:], ones_sb[:], al_v[:, c, :],
                             start=True, stop=True)
            nc.scalar.activation(abc_v[:, c, :], ps_b[:], AF.Copy)

        # ---- F gate (PE work overlapping the softmax/ACT tail) ----
        psl = gate_gemm("Wfx", "Wfh")
        for i in range(JT):
            nc.scalar.activation(fT[:, i, :], psl[i][:], AF.Sigmoid,
                                 bias=bias_sb["bF"][:, i:i + 1])

        # ---- U = x @ Wux; ut = tanh(U + u_h + bU) ----
        # u_h[i] = sum_k hs[:, i, :, k] * abc[:, :, k], added into the open
        # U psums (DVE chain overlaps the O-gate GEMMs below)
        ps_u = [ps.tile([P, BS], F32, name=f"psu{i}", tag="ps")
                for i in range(JT)]
        for j in range(JT):
            wt = wpool.tile([P, H], BF16, tag="w")
            nc.sync.dma_start(wt[:], dram["Wux"].ap()[j * P:(j + 1) * P, :])
            for i in range(JT):
                nc.tensor.matmul(ps_u[i][:], wt[:, i * P:(i + 1) * P],
                                 xT_sb[:, j, :], start=(j == 0), stop=(j == JT - 1))
        for i in range(JT):
            pr = pr_p.tile([P, BS, K], BF16, tag="pr")
            nc.vector.tensor_tensor(pr[:], hs[:, i, :, :], abc[:], ALU.mult)
            uh_t = tmp_p.tile([P, BS], F32, tag="uht")
            nc.vector.tensor_reduce(uh_t[:], pr[:], mybir.AxisListType.X,
                                    ALU.add)
            nc.vector.tensor_add(ps_u[i][:], ps_u[i][:], uh_t[:])
            nc.scalar.activation(utT[:, i, :], ps_u[i][:], AF.Tanh,
                                 bias=bias_sb["bU"][:, i:i + 1])

        # ---- O gate (PE work overlapping the u_h DVE chain) ----
        psl = gate_gemm("Wox", "Woh")
        for i in range(JT):
            nc.scalar.activation(oT[:, i, :], psl[i][:], AF.Sigmoid,
                                 bias=bias_sb["bO"][:, i:i + 1])

        # ---- transpose f_s, o_s, ut to natural layout; final state math ----
        for t in range(NT):
            fN = nf_p.tile([P, H], BF16, tag="fN")
            oN = nf_p.tile([P, H], BF16, tag="oN")
            uN = nf_p.tile([P, H], BF16, tag="uN")
            for src, dst in ((fT, fN), (oT, oN), (utT, uN)):
                for i in range(JT):
                    ps_tr = ps.tile([P, P], BF16, tag="ps")
                    nc.tensor.matmul(ps_tr[:], src[:, i, t * P:(t + 1) * P],
                                     id_bf[:], is_transpose=True,
                                     start=True, stop=True)
                    nc.scalar.activation(dst[:, i * P:(i + 1) * P], ps_tr[:],
                                         AF.Copy)
            clt = cl_p.tile([P, H], F32, tag="cl")
            nc.sync.dma_start(clt[:], dram["cl"].ap()[t * P:(t + 1) * P, :])
            # cell = (c_last - ut) * f + ut ; hidden = tanh(cell) * o
            diff = tmp_p.tile([P, H], F32, tag="diff")
            nc.vector.tensor_sub(diff[:], clt[:], uN[:])
            cell = out_p.tile([P, H], F32, tag="o")
            nc.vector.tensor_tensor(cell[:], diff[:], fN[:], ALU.mult)
            nc.vector.tensor_add(cell[:], cell[:], uN[:])
            th = tmp_p.tile([P, H], BF16, tag="th")
            nc.scalar.activation(th[:], cell[:], AF.Tanh)
            hid = out_p.tile([P, H], F32, tag="o")
            nc.vector.tensor_tensor(hid[:], th[:], oN[:], ALU.mult)
            nc.sync.dma_start(cel_o.ap()[t * P:(t + 1) * P, :], cell[:])
            nc.sync.dma_start(hid_o.ap()[t * P:(t + 1) * P, :], hid[:])


def kernel(**inputs):
    x = np.asarray(inputs["x"], dtype=np.float32)
    hiddens = np.asarray(inputs["hiddens"], dtype=np.float32)
    cells = np.asarray(inputs["cells"], dtype=np.float32)

    if "nc" not in _CACHE:
        _CACHE["nc"] = _build()
    nc = _CACHE["nc"]

    wb = {}
    for w in ("Wfx", "Wox", "Wix", "Wux", "Wfh", "Woh", "Wih"):
        wb[w] = np.asarray(inputs[w], dtype=np.float32).astype(bf16)
    Wk_b = np.asarray(inputs["Wk"], dtype=np.float32).astype(bf16)
    attnW_b = np.asarray(inputs["attnW"], dtype=np.float32).astype(bf16)
    attnWu_b = np.asarray(inputs["attnWu"], dtype=np.float32).astype(bf16).reshape(A, 1)
    attnb_f = np.asarray(inputs["attnb"], dtype=np.float32).reshape(A, 1)

    def fold_bias(b):
        return np.ascontiguousarray(
            np.asarray(b, dtype=np.float32).reshape(JT, P).T)

    bI = fold_bias(np.asarray(inputs["bix"], np.float32) + np.asarray(inputs["bih"], np.float32))
    bF = fold_bias(np.asarray(inputs["bfx"], np.float32) + np.asarray(inputs["bfh"], np.float32))
    bO = fold_bias(np.asarray(inputs["box"], np.float32) + np.asarray(inputs["boh"], np.float32))
    bU = fold_bias(np.asarray(inputs["bux"], np.float32))
    bkr = np.ascontiguousarray(
        np.asarray(inputs["bk"], np.float32).reshape(K, JT, P).transpose(2, 0, 1))
    ones = np.ones((1, P), dtype=bf16)

    x_b = x.astype(bf16)
    h_b = hiddens.astype(bf16)
    c_last = cells[K - 1]

    in_maps = []
    for c in range(NCORES):
        sl = slice(c * BS, (c + 1) * BS)
        m = {
            "xT": np.ascontiguousarray(x_b[sl].T),
            "hT": np.ascontiguousarray(h_b[:, sl].transpose(0, 2, 1)),
            "cl": np.ascontiguousarray(c_last[sl]),
            "Wk": Wk_b, "attnW": attnW_b, "attnWu": attnWu_b,
            "attnb": attnb_f, "bI": bI, "bF": bF, "bO": bO, "bU": bU,
            "bkr": bkr, "ones": ones,
        }
        m.update(wb)
        in_maps.append(m)

    res = run_bass_kernel_spmd(nc, in_maps, list(range(NCORES)))
    hidden = np.empty((B, H), np.float32)
    cell = np.empty((B, H), np.float32)
    for c in range(NCORES):
        sl = slice(c * BS, (c + 1) * BS)
        hidden[sl] = res.results[c]["hidden"]
        cell[sl] = res.results[c]["cell"]
    return hidden, cell
